# revision 10
# baseline (speedup 1.0000x reference)
"""SNN recurrent layer (Linear + leaky-integrate-and-fire scan) on 8 trn2 NeuronCores.

v4 design (pure data parallel over batch; 32 batches/core):
  - Host splits X into bf16 hi/lo halves and pre-transposes to the PE-stationary
    layout [b, ttile, kchunk, i, hl, t]; W likewise to [k, i, hl, o]. The GEMM
    h = X @ W.T runs as 3 bf16 passes (hi*w_hi + lo*w_hi + hi*w_lo), exact to
    ~2^-16 relative, accumulated in one fp32 PSUM bank per (b, ttile):
    18 matmuls of ap=400 (stream 167ns >= bf16 LDW 97ns, so weight loads hide).
  - No PE transposes: X arrives transposed from host.
  - h tiles [t<=128, 400] are scattered (SBUF->SBUF DMA, 512B descriptors) into
    the scan layout hseg[(oc,b) partition, t, o' (o-groups of 128, last padded)].
  - The syn scan is fused into the membrane loop: per timestep two DVE ops over
    all 128 partitions x 128 lanes:
       mem' = (mem <= 1) ? (beta*mem + syn) : 0      (custom DVE op)
       syn' = alpha*syn + h_t                        (scalar_tensor_tensor)
  - Spikes (mem > 1) are extracted in batches of 20 steps as uint8 and DMA'd to
    DRAM in [oc, b, t, o'] layout (2.5KB descriptors); host reassembles and
    widens to f32.
"""

import numpy as np

ALPHA = 0.9
BETA = 0.85

B_FULL, T_FULL, I_FULL, O_FULL = 256, 500, 700, 400
NCORES = 8
P = 128
KC = 6                      # i-chunks of 128 (i padded 700 -> 768)
IP = KC * P                 # 768
TPAD = 512                  # t padded 500 -> 512 (4 tiles of 128)
TT_LENS = [128, 128, 128, 116]
MEMK = 20
OPP = 128                   # o' lanes per partition-group (padded; last group 16 real)

_CACHE = {}


# --------------------------------------------------------------------------- #
# Custom DVE op: one fused membrane update step.
#   out = select(mem <= 1, beta*mem + syn, 0)
# --------------------------------------------------------------------------- #
def _register_memstep():
    import concourse.dve_ops as dvo
    from concourse.dve_spec import Spec, Src0, Src1, C0, Zero, One, select

    for op in dvo.OPS:
        if op.name == "SNN_MEMSTEP_ANT":
            return op

    def _ref(in0, in1, s0, s1, imm2):
        a = (in0.astype(np.float32) * np.float32(s0) + in1).astype(np.float32)
        return np.where(in0 <= 1.0, a, np.float32(0.0)).astype(np.float32)

    spec = Spec(body=select(Src0 <= One, Src0 * C0 + Src1, Zero), reference=_ref)

    def _append(op):
        dvo.OPS.append(op)
        dvo.CUSTOM_DVE_SPECS[op.name] = op.spec
        dvo._SUB_OPCODE_FOR_NAME[op.name] = dvo._CUSTOM_DVE_ROW_BASE + len(dvo.OPS) - 1

    # Two-phase registration: learn the uops shas from the pin-check error.
    import re as _re

    probe = dvo.DveOp("SNN_MEMSTEP_ANT", spec, subdim=False, uops_sha={})
    _append(probe)
    shas = {}
    for ver in ("v3", "v4"):
        try:
            probe.compile(ver)
            shas[ver] = probe.uops_sha[ver]
        except ValueError as e:
            m = _re.search(r'uops_sha\["(v\d)"\]="([0-9a-f]+)"', str(e))
            shas[m.group(1)] = m.group(2)
    dvo.OPS.remove(probe)
    del dvo._SUB_OPCODE_FOR_NAME[probe.name]
    final = dvo.DveOp("SNN_MEMSTEP_ANT", spec, subdim=False, uops_sha=shas)
    _append(final)
    return final


# --------------------------------------------------------------------------- #
# Program builder (per-core SPMD program).
# --------------------------------------------------------------------------- #
def build_program(B_L, T, I, O, spike_on_act=True):
    import concourse.bass as bass
    import concourse.bacc as bacc
    import concourse.mybir as mybir
    import concourse.tile as tile

    MEMSTEP = _register_memstep()

    assert B_L == 32 and T == 500 and I == 700 and O == 400
    NTT = len(TT_LENS)
    TT_STARTS = np.cumsum([0] + TT_LENS).tolist()
    # o-column slices per partition-group (padded to OPP lanes each)
    OSL = [(0, 128), (128, 256), (256, 384), (384, 400)]

    f32 = mybir.dt.float32
    bf16 = mybir.dt.bfloat16
    u8 = mybir.dt.uint8

    nc = bacc.Bacc(
        "TRN2",
        target_bir_lowering=False,
        debug=False,
        enable_asserts=False,
        num_devices=1,
    )

    x_d = nc.dram_tensor("x", [B_L, NTT, KC, P, 2, P], bf16, kind="ExternalInput").ap()
    w_d = nc.dram_tensor("w", [KC, P, 2, O], bf16, kind="ExternalInput").ap()
    out_d = nc.dram_tensor("out", [4, B_L, T, OPP], u8, kind="ExternalOutput").ap()

    def seg_of(t):
        for s in range(NTT):
            if t < TT_STARTS[s + 1]:
                return s, t - TT_STARTS[s]
        raise AssertionError

    with tile.TileContext(nc) as tc:
        with (
            tc.tile_pool(name="persist", bufs=1) as pp,
            tc.tile_pool(name="xp", bufs=3) as xp,
            tc.tile_pool(name="hp", bufs=3) as hp,
            tc.tile_pool(name="stp", bufs=3) as stp,
            tc.tile_pool(name="psp", bufs=3, space=bass.MemorySpace.PSUM) as psp,
        ):
            # ---------------- persistent tiles ----------------
            wt = pp.tile([P, KC, 2, O], bf16)
            nc.sync.dma_start(wt[:, :, :, :], w_d.transpose([1, 0, 2, 3]))
            syn = pp.tile([P, OPP], f32)
            nc.vector.memset(syn[:, :], 0.0)
            ring = pp.tile([P, MEMK + 1, OPP], f32)
            nc.vector.memset(ring[:, 0, :], 0.0)
            neg1 = pp.tile([P, 1], f32)
            nc.vector.memset(neg1[:, :], -1.0)
            hseg = [
                pp.tile([P, 128, OPP], f32, name=f"hseg{i}", tag=f"hseg{i}")
                for i in range(2)
            ]

            def emit_memseg(s):
                Ts = TT_LENS[s]
                t0 = TT_STARTS[s]
                hb = hseg[s % 2]
                for t in range(t0, t0 + Ts):
                    tl = t - t0
                    j = t % MEMK
                    # mem_{t+1} = (beta*mem_t + syn_t) * [mem_t <= 1]
                    nc.vector._custom_dve(
                        MEMSTEP,
                        out=ring[:, j + 1, :],
                        in0=ring[:, j, :],
                        in1=syn[:, :],
                        s0=BETA,
                    )
                    # syn_{t+1} = alpha*syn_t + h_t
                    nc.vector.scalar_tensor_tensor(
                        syn[:, :],
                        syn[:, :],
                        ALPHA,
                        hb[:, tl, :],
                        op0=mybir.AluOpType.mult,
                        op1=mybir.AluOpType.add,
                    )
                    if j == MEMK - 1:
                        tb0 = t - (MEMK - 1)
                        stage = stp.tile([P, MEMK, OPP], u8, tag="stage")
                        if spike_on_act:
                            # u8(Sign(mem - 1)): saturates -1 -> 0
                            nc.scalar.sign(
                                stage[:, :, :], ring[:, 0:MEMK, :], bias=neg1[:, 0:1]
                            )
                        else:
                            nc.vector.tensor_scalar(
                                stage[:, :, :],
                                ring[:, 0:MEMK, :],
                                1.0,
                                None,
                                op0=mybir.AluOpType.is_gt,
                            )
                        for oc in range(4):
                            nc.sync.dma_start(
                                out_d[oc, :, tb0:tb0 + MEMK, :],
                                stage[oc * 32:(oc + 1) * 32, :, :],
                            )
                        nc.vector.tensor_copy(ring[:, 0, :], ring[:, MEMK, :])

            # ---------------- main pipeline (ttile-outer) ----------------
            for s in range(NTT):
                Ts = TT_LENS[s]
                for b in range(B_L):
                    xb = xp.tile([P, KC, 2, P], bf16, tag="xb")
                    nc.sync.dma_start(xb[:, :, :, :], x_d[b, s].transpose([1, 0, 2, 3]))

                    ps = psp.tile([P, O], f32, tag="ps")
                    nmm = 3 * KC
                    i = 0
                    for hl_x, hl_w in ((0, 0), (1, 0), (0, 1)):
                        for k in range(KC):
                            nc.tensor.matmul(
                                ps[:, :],
                                xb[:, k, hl_x, :],
                                wt[:, k, hl_w, :],
                                start=(i == 0),
                                stop=(i == nmm - 1),
                            )
                            i += 1

                    h_sb = hp.tile([P, O], f32, tag="h_sb")
                    nc.scalar.copy(h_sb[:, :], ps[:, :])

                    hb = hseg[s % 2]
                    for oc in range(4):
                        lo, hi = OSL[oc]
                        pr = oc * 32 + b
                        nc.sync.dma_start(
                            hb[pr:pr + 1, 0:Ts, 0:hi - lo],
                            h_sb[0:Ts, lo:hi],
                        )
                emit_memseg(s)

    nc.compile()
    return nc, {}


# --------------------------------------------------------------------------- #
# Host-side pre/post processing
# --------------------------------------------------------------------------- #
def _bf16_rne(a_f32):
    """Round-to-nearest-even bf16, as uint16 bits."""
    u = a_f32.view(np.uint32)
    r = (u + np.uint32(0x7FFF) + ((u >> np.uint32(16)) & np.uint32(1))) >> np.uint32(16)
    return r.astype(np.uint16)


def _prep_x_core(xc):
    """[32, 500, 700] f32 -> [32, 4, 6, 128, 2, 128] bf16 (as uint16)."""
    B_L = xc.shape[0]
    xt = np.zeros((B_L, IP, TPAD), dtype=np.float32)
    xt[:, :I_FULL, :T_FULL] = xc.transpose(0, 2, 1)
    hi_u = _bf16_rne(xt)
    hi_f = (hi_u.astype(np.uint32) << np.uint32(16)).view(np.float32)
    lo_u = _bf16_rne(xt - hi_f)
    # [b, k, i, tt, t] -> [b, tt, k, i, hl, t]
    hi_u = hi_u.reshape(B_L, KC, P, 4, P)
    lo_u = lo_u.reshape(B_L, KC, P, 4, P)
    cat = np.stack([hi_u, lo_u], axis=4)          # [b, k, i, tt, 2, t]
    cat = cat.transpose(0, 3, 1, 2, 4, 5)         # [b, tt, k, i, 2, t]
    return np.ascontiguousarray(cat)


def _prep_w(W):
    """[400, 700] f32 -> [6, 128, 2, 400] bf16 (as uint16)."""
    wt = np.zeros((IP, O_FULL), dtype=np.float32)
    wt[:I_FULL, :] = W.T
    hi_u = _bf16_rne(wt)
    hi_f = (hi_u.astype(np.uint32) << np.uint32(16)).view(np.float32)
    lo_u = _bf16_rne(wt - hi_f)
    hi_u = hi_u.reshape(KC, P, O_FULL)
    lo_u = lo_u.reshape(KC, P, O_FULL)
    return np.ascontiguousarray(np.stack([hi_u, lo_u], axis=2))


def kernel(inputs: np.ndarray, W: np.ndarray, nb_steps) -> np.ndarray:
    import ml_dtypes
    from concourse.bass_utils import run_bass_kernel_spmd

    B, T, I = inputs.shape
    O = W.shape[0]
    assert (B, T, I, O) == (B_FULL, T_FULL, I_FULL, O_FULL), (B, T, I, O)
    assert int(nb_steps) == T

    key = (B, T, I, O)
    if key not in _CACHE:
        _CACHE[key] = build_program(B // NCORES, T, I, O)
    nc, meta = _CACHE[key]

    B_L = B // NCORES
    x = np.ascontiguousarray(inputs, dtype=np.float32)
    w_cat = _prep_w(np.ascontiguousarray(W, dtype=np.float32)).view(ml_dtypes.bfloat16)
    in_maps = []
    for c in range(NCORES):
        xc = _prep_x_core(x[c * B_L:(c + 1) * B_L]).view(ml_dtypes.bfloat16)
        in_maps.append({"x": xc, "w": w_cat})
    results = run_bass_kernel_spmd(nc, in_maps, core_ids=list(range(NCORES)))

    out = np.empty((B, T, O), dtype=np.float32)
    for c in range(NCORES):
        buf = results.results[c]["out"]           # [4, 32, 500, 128] u8
        ov = out[c * B_L:(c + 1) * B_L]
        for oc in range(4):
            lo, hi = oc * 128, min((oc + 1) * 128, O)
            ov[:, :, lo:hi] = buf[oc, :, :, 0:hi - lo]
    return out


# revision 11
# speedup vs baseline: 2.7108x; 2.7108x over previous
"""SNN recurrent layer (Linear + leaky-integrate-and-fire scan) on 8 trn2 NeuronCores.

v5 design (pure data parallel over batch; 32 batches/core):
  - Host splits X into bf16 hi/lo halves and lays it out as
    [tblock, k, i, hl, t16, b32]; W as [k, i, hl, o]. The GEMM h = X @ W.T runs
    W-stationary as 3 bf16 passes (hi*w_hi + lo*w_hi + hi*w_lo; exact to
    ~2^-16), producing h^T tiles [o' (o-tile of <=128), (t16, b32)=512] in one
    fp32 PSUM bank: 18 matmuls of ap=512 per (tblock, o-tile); bf16 weight
    loads (97ns) hide under the 216ns streams.
  - The PSUM->SBUF Activation copy IS the layout shuffle: h^T lands directly in
    the membrane-loop layout hseg[o' partition, o-tile, t, b] (partition-
    aligned, no DMA scatter at all).
  - The syn scan is fused into the membrane loop: per timestep two DVE ops over
    [128, 4, 32] lanes:
       mem' = (mem <= 1) ? (beta*mem + syn) : 0      (custom DVE op)
       syn' = alpha*syn + h_t                        (scalar_tensor_tensor)
  - Spikes (mem > 1) extracted as uint8 in 20-step batches on DVE, DMA'd to
    DRAM [o-tile, o', t, b]; host reassembles/widens to f32 [B, T, O].
  - Membrane-loop instructions for segment s are emitted interleaved with the
    GEMM blocks of segment s+1 so the single sync DMA queue never stalls
    behind spike-gated output DMAs.
"""

import numpy as np

ALPHA = 0.9
BETA = 0.85

B_FULL, T_FULL, I_FULL, O_FULL = 256, 500, 700, 400
NCORES = 8
P = 128
KC = 6                      # i-chunks of 128 (i padded 700 -> 768)
IP = KC * P                 # 768
TPAD = 512                  # t padded 500 -> 512: 32 blocks of 16
TBLK = 16
NBLK = TPAD // TBLK         # 32
B_L = 32
SEG = 128                   # mem-loop segment (8 blocks)
NSEG = 4
SEG_LENS = [128, 128, 128, 116]
MEMK = 20
OT_SL = [(0, 128), (128, 256), (256, 384), (384, 400)]

_CACHE = {}


# --------------------------------------------------------------------------- #
# Custom DVE op: one fused membrane update step.
#   out = select(mem <= 1, beta*mem + syn, 0)
# --------------------------------------------------------------------------- #
def _register_memstep():
    import concourse.dve_ops as dvo
    from concourse.dve_spec import Spec, Src0, Src1, C0, Zero, One, select

    for op in dvo.OPS:
        if op.name == "SNN_MEMSTEP_ANT":
            return op

    def _ref(in0, in1, s0, s1, imm2):
        a = (in0.astype(np.float32) * np.float32(s0) + in1).astype(np.float32)
        return np.where(in0 <= 1.0, a, np.float32(0.0)).astype(np.float32)

    spec = Spec(body=select(Src0 <= One, Src0 * C0 + Src1, Zero), reference=_ref)

    def _append(op):
        dvo.OPS.append(op)
        dvo.CUSTOM_DVE_SPECS[op.name] = op.spec
        dvo._SUB_OPCODE_FOR_NAME[op.name] = dvo._CUSTOM_DVE_ROW_BASE + len(dvo.OPS) - 1

    import re as _re

    probe = dvo.DveOp("SNN_MEMSTEP_ANT", spec, subdim=False, uops_sha={})
    _append(probe)
    shas = {}
    for ver in ("v3", "v4"):
        try:
            probe.compile(ver)
            shas[ver] = probe.uops_sha[ver]
        except ValueError as e:
            m = _re.search(r'uops_sha\["(v\d)"\]="([0-9a-f]+)"', str(e))
            shas[m.group(1)] = m.group(2)
    dvo.OPS.remove(probe)
    del dvo._SUB_OPCODE_FOR_NAME[probe.name]
    final = dvo.DveOp("SNN_MEMSTEP_ANT", spec, subdim=False, uops_sha=shas)
    _append(final)
    return final


# --------------------------------------------------------------------------- #
# Program builder (per-core SPMD program).
# --------------------------------------------------------------------------- #
def build_program(spike_on_act=False):
    import concourse.bass as bass
    import concourse.bacc as bacc
    import concourse.mybir as mybir
    import concourse.tile as tile

    MEMSTEP = _register_memstep()

    f32 = mybir.dt.float32
    bf16 = mybir.dt.bfloat16
    u8 = mybir.dt.uint8
    T = T_FULL
    O = O_FULL

    nc = bacc.Bacc(
        "TRN2",
        target_bir_lowering=False,
        debug=False,
        enable_asserts=False,
        num_devices=1,
    )

    x_d = nc.dram_tensor(
        "x", [NBLK, KC, P, 2, TBLK, B_L], bf16, kind="ExternalInput"
    ).ap()
    w_d = nc.dram_tensor("w", [KC, P, 2, O], bf16, kind="ExternalInput").ap()
    out_d = nc.dram_tensor("out", [4, P, T, B_L], u8, kind="ExternalOutput").ap()

    with tile.TileContext(nc) as tc:
        with (
            tc.tile_pool(name="persist", bufs=1) as pp,
            tc.tile_pool(name="xp", bufs=2) as xp,
            tc.tile_pool(name="stp", bufs=3) as stp,
            tc.tile_pool(name="psp", bufs=3, space=bass.MemorySpace.PSUM) as psp,
        ):
            # ---------------- persistent tiles ----------------
            wt = pp.tile([P, KC, 2, O], bf16)
            nc.sync.dma_start(wt[:, :, :, :], w_d.transpose([1, 0, 2, 3]))
            syn = pp.tile([P, 4, B_L], f32)
            nc.vector.memset(syn[:, :, :], 0.0)
            ring = pp.tile([P, MEMK + 1, 4, B_L], f32)
            nc.vector.memset(ring[:, 0, :, :], 0.0)
            hseg = [
                pp.tile([P, 4, SEG, B_L], f32, name=f"hseg{i}", tag=f"hseg{i}")
                for i in range(2)
            ]

            # ---- per-seg mem-loop emission, sliced for interleaving ----
            def memseg_slice(s, tl0, tl1):
                """Emit mem-loop steps for local t in [tl0, tl1) of segment s."""
                hb = hseg[s % 2]
                t0 = s * SEG
                for tl in range(tl0, min(tl1, SEG_LENS[s])):
                    t = t0 + tl
                    j = t % MEMK
                    nc.vector._custom_dve(
                        MEMSTEP,
                        out=ring[:, j + 1, :, :],
                        in0=ring[:, j, :, :],
                        in1=syn[:, :, :],
                        s0=BETA,
                    )
                    nc.vector.scalar_tensor_tensor(
                        syn[:, :, :],
                        syn[:, :, :],
                        ALPHA,
                        hb[:, :, tl, :],
                        op0=mybir.AluOpType.mult,
                        op1=mybir.AluOpType.add,
                    )
                    if j == MEMK - 1:
                        tb0 = t - (MEMK - 1)
                        stage = stp.tile([P, MEMK, 4, B_L], u8, tag="stage")
                        nc.vector.tensor_scalar(
                            stage[:, :, :, :],
                            ring[:, 0:MEMK, :, :],
                            1.0,
                            None,
                            op0=mybir.AluOpType.is_gt,
                        )
                        for ot in range(4):
                            nc.sync.dma_start(
                                out_d[ot, :, tb0:tb0 + MEMK, :],
                                stage[:, :, ot, :],
                            )
                        nc.vector.tensor_copy(ring[:, 0, :, :], ring[:, MEMK, :, :])

            def gemm_block(blk):
                """One t-block: load x, 4 o-tiles x 18 matmuls, copies to hseg."""
                s = blk // 8
                tl = (blk % 8) * TBLK
                xb = xp.tile([P, KC, 2, TBLK, B_L], bf16, tag="xb")
                for k in range(KC):
                    nc.sync.dma_start(xb[:, k, :, :, :], x_d[blk, k])
                hb = hseg[s % 2]
                for ot in range(4):
                    lo, hi = OT_SL[ot]
                    m = hi - lo
                    ps = psp.tile([P, TBLK * B_L], f32, tag="ps")
                    i = 0
                    for hl_x, hl_w in ((0, 0), (1, 0), (0, 1)):
                        for k in range(KC):
                            nc.tensor.matmul(
                                ps[0:m, :],
                                wt[:, k, hl_w, lo:hi],
                                xb[:, k, hl_x, :, :],
                                start=(i == 0),
                                stop=(i == 17),
                            )
                            i += 1
                    nc.scalar.copy(
                        hb[0:m, ot, tl:tl + TBLK, :],
                        ps[0:m, :].rearrange("p (t b) -> p t b", b=B_L),
                    )

            # ---------------- main pipeline ----------------
            # seg s GEMM blocks run interleaved with memseg(s-1) slices.
            for s in range(NSEG):
                for bl in range(8):
                    gemm_block(s * 8 + bl)
                    if s > 0:
                        memseg_slice(s - 1, bl * TBLK, (bl + 1) * TBLK)
            memseg_slice(NSEG - 1, 0, SEG)

    nc.compile()
    return nc, {}


# --------------------------------------------------------------------------- #
# Host-side pre/post processing
# --------------------------------------------------------------------------- #
def _bf16_rne(a_f32):
    """Round-to-nearest-even bf16, as uint16 bits."""
    u = a_f32.view(np.uint32)
    r = (u + np.uint32(0x7FFF) + ((u >> np.uint32(16)) & np.uint32(1))) >> np.uint32(16)
    return r.astype(np.uint16)


def _prep_x_core(xc):
    """[32, 500, 700] f32 -> [32blk, 6k, 128i, 2hl, 16t, 32b] bf16 bits."""
    xt = np.zeros((B_L, TPAD, IP), dtype=np.float32)
    xt[:, :T_FULL, :I_FULL] = xc
    hi_u = _bf16_rne(xt)
    hi_f = (hi_u.astype(np.uint32) << np.uint32(16)).view(np.float32)
    lo_u = _bf16_rne(xt - hi_f)
    # [b, (blk, t), (k, i)] -> [blk, k, i, hl, t, b]
    hi_u = hi_u.reshape(B_L, NBLK, TBLK, KC, P)
    lo_u = lo_u.reshape(B_L, NBLK, TBLK, KC, P)
    cat = np.stack([hi_u, lo_u], axis=3)          # [b, blk, t, hl, k, i]
    cat = cat.transpose(1, 4, 5, 3, 2, 0)         # [blk, k, i, hl, t, b]
    return np.ascontiguousarray(cat)


def _prep_w(W):
    """[400, 700] f32 -> [6, 128, 2, 400] bf16 bits."""
    wt = np.zeros((IP, O_FULL), dtype=np.float32)
    wt[:I_FULL, :] = W.T
    hi_u = _bf16_rne(wt)
    hi_f = (hi_u.astype(np.uint32) << np.uint32(16)).view(np.float32)
    lo_u = _bf16_rne(wt - hi_f)
    hi_u = hi_u.reshape(KC, P, O_FULL)
    lo_u = lo_u.reshape(KC, P, O_FULL)
    return np.ascontiguousarray(np.stack([hi_u, lo_u], axis=2))


def kernel(inputs: np.ndarray, W: np.ndarray, nb_steps) -> np.ndarray:
    import ml_dtypes
    from concourse.bass_utils import run_bass_kernel_spmd

    B, T, I = inputs.shape
    O = W.shape[0]
    assert (B, T, I, O) == (B_FULL, T_FULL, I_FULL, O_FULL), (B, T, I, O)
    assert int(nb_steps) == T

    if "prog" not in _CACHE:
        _CACHE["prog"] = build_program()
    nc, meta = _CACHE["prog"]

    x = np.ascontiguousarray(inputs, dtype=np.float32)
    w_cat = _prep_w(np.ascontiguousarray(W, dtype=np.float32)).view(ml_dtypes.bfloat16)
    in_maps = []
    for c in range(NCORES):
        xc = _prep_x_core(x[c * B_L:(c + 1) * B_L]).view(ml_dtypes.bfloat16)
        in_maps.append({"x": xc, "w": w_cat})
    results = run_bass_kernel_spmd(nc, in_maps, core_ids=list(range(NCORES)))

    out = np.empty((B, T, O), dtype=np.float32)
    for c in range(NCORES):
        buf = results.results[c]["out"]           # [4, 128, 500, 32] u8
        ov = out[c * B_L:(c + 1) * B_L]
        for ot in range(4):
            lo, hi = OT_SL[ot]
            ov[:, :, lo:hi] = buf[ot, 0:hi - lo].transpose(2, 1, 0)
    return out
